# revision 17
# baseline (speedup 1.0000x reference)
"""nn_CrossAttention kernel for 8 Trainium2 NeuronCores.

Sharding: data-parallel over batch B=8, one batch element per core, no
collectives. All heavy matmuls run in fp8e4 (e4m3) with power-of-2 scale
compensation; C=512-contraction projections, attention O/row-sum matmuls
(key-block pairs) and output projections (head pairs) use DoubleRow perf
mode (K=256 at 0.5 cyc/col). Phase A (x@W1 / y@W2, the fp32 residual
path) uses float32r at full rate. Wq2@Wk2^T is folded into a single
64x64 matrix so branch-2 scores contract raw k2/q2 at K=64. Softmax
denominators come from ones-matmuls riding spare PSUM rows of the O2
accumulator; normalization is a tensor-tensor divide against a
partition-broadcast row. Branches are interleaved per head so PSUM fits
in 8 banks; softmax exp on the Act engine (~127us) is the roofline and
all other engine work (PE/DVE/Pool/DMA) is hidden under it.
"""
import sys

sys.path.insert(0, "/opt/trn_rl_repo")

import numpy as np
import ml_dtypes

import concourse.bass as bass
import concourse.tile as tile
from concourse import bacc, mybir, bass2jax

F32 = mybir.dt.float32
F32R = mybir.dt.float32r
BF16 = mybir.dt.bfloat16
FP8 = mybir.dt.float8e4
DR = mybir.MatmulPerfMode.DoubleRow
EXP = mybir.ActivationFunctionType.Exp
MULT = mybir.AluOpType.mult
ADD = mybir.AluOpType.add
DIV = mybir.AluOpType.divide

N_CORES = 8
H, D = 8, 64          # heads, head_dim
D2 = 2 * D            # 128
NT = 1024             # tokens
C = 512               # model dim
KB = 8                # key blocks of 128
NP = KB // 2          # key-block pairs
SCALE = D ** -0.5

# power-of-2 quantization scales
SX = 16.0             # xcb8/ycb8 = xc * SX
SW = 16.0             # projection weights * SW
SK = 16.0             # knew8 = k * SK
SQ1 = 64.0            # q1p8 = q1 * SQ1
SQ2 = 16.0            # q2 raw * SQ2
SM2 = 64.0            # M2^T * SM2
SKP = 64.0            # k2p' * SKP
SWP = 16.0            # Wp1/Wp2 * SWP
SIDENT = 64.0         # ident * SIDENT (must equal 256*SWP/SQ1)
SE = 256.0 * SWP      # scale of phase-E psum (= SQ1*SIDENT)
EXPSCALE = SCALE / (SQ1 * SK)   # == SCALE / (SQ2 * SKP)


def _build(nc):
    dram = {}
    def din(name, shape, dt):
        dram[name] = nc.dram_tensor(name, shape, dt, kind="ExternalInput").ap()
    din("xT", [84, NT], F32R)
    din("yT", [50, NT], F32R)
    din("W1", [84, C], F32R)
    din("W2", [50, C], F32R)
    for g in range(2):
        din(f"w1v8_{g}", [128, 2, C], FP8)
        din(f"w2v8_{g}", [128, 2, C], FP8)
        din(f"w1k8_{g}", [128, 2, C], FP8)
        din(f"w2k8_{g}", [128, 2, C], FP8)
        din(f"w1q8_{g}", [128, 2, 1024], FP8)
        din(f"w2q8_{g}", [128, 2, C], FP8)
    din("m2t8", [64, 64], FP8)
    for p in range(4):
        din(f"wp1p8_{p}", [128, 2, C], FP8)
        din(f"wp2p8_{p}", [64, 2, C], FP8)
    din("ident8", [128, 128], FP8)
    din("bp1", [C], F32)
    din("bp2", [C], F32)
    outT = nc.dram_tensor("outT", [2 * C, NT], F32, kind="ExternalOutput").ap()

    with tile.TileContext(nc) as tc:
        _body(tc, nc, dram, outT)
    return dram, outT


def _body(tc, nc, dram, outT):
    from contextlib import ExitStack
    ctx = ExitStack()
    with ctx:
        wts = ctx.enter_context(tc.tile_pool(name="wts", bufs=1))
        acts = ctx.enter_context(tc.tile_pool(name="acts", bufs=1))

        def load(pool, name, shape, dt, src_ap=None):
            t = pool.tile(shape, dt, tag=name, name=name)
            nc.sync.dma_start(out=t, in_=dram[name] if src_ap is None else src_ap)
            return t

        # ---- persistent weights ----
        w1 = load(wts, "W1", [84, C], F32R)
        w2 = load(wts, "W2", [50, C], F32R)
        w1v8 = [load(wts, f"w1v8_{g}", [128, 2, C], FP8) for g in range(2)]
        w2v8 = [load(wts, f"w2v8_{g}", [128, 2, C], FP8) for g in range(2)]
        w1k8 = [load(wts, f"w1k8_{g}", [128, 2, C], FP8) for g in range(2)]
        w2k8 = [load(wts, f"w2k8_{g}", [128, 2, C], FP8) for g in range(2)]
        w1q8 = [load(wts, f"w1q8_{g}", [128, 2, 1024], FP8) for g in range(2)]
        w2q8 = [load(wts, f"w2q8_{g}", [128, 2, C], FP8) for g in range(2)]
        m2t8 = wts.tile([128, 64], FP8, tag="m2t8", name="m2t8")
        nc.sync.dma_start(out=m2t8[64:128, :], in_=dram["m2t8"])
        wp1p8 = [load(wts, f"wp1p8_{p}", [128, 2, C], FP8) for p in range(4)]
        wp2p8 = [load(wts, f"wp2p8_{p}", [64, 2, C], FP8) for p in range(4)]
        ident8 = load(wts, "ident8", [128, 128], FP8)
        bp1 = wts.tile([128, 4], F32, tag="bp1", name="bp1")
        nc.sync.dma_start(out=bp1, in_=dram["bp1"].rearrange("(j p) -> p j", j=4))
        bp2 = wts.tile([128, 4], F32, tag="bp2", name="bp2")
        nc.sync.dma_start(out=bp2, in_=dram["bp2"].rearrange("(j p) -> p j", j=4))
        ones8 = wts.tile([128, 2, 32], FP8, tag="ones8", name="ones8")
        nc.vector.memset(ones8, 1.0)

        # ---- persistent activations ----
        xc = [acts.tile([128, NT], F32, tag=f"xc{j}", name=f"xc{j}") for j in range(4)]
        yc = [acts.tile([128, NT], F32, tag=f"yc{j}", name=f"yc{j}") for j in range(4)]
        xcb8 = acts.tile([128, 4, NT], FP8, tag="xcb8", name="xcb8")
        ycb8 = acts.tile([128, 4, NT], FP8, tag="ycb8", name="ycb8")
        knew8 = [acts.tile([128, NT], FP8, tag=f"kn{h}", name=f"kn{h}") for h in range(H)]
        q1p8 = [acts.tile([128, NT], FP8, tag=f"q1p{h}", name=f"q1p{h}") for h in range(H)]
        q2p8 = [acts.tile([128, NT], FP8, tag=f"q2p{p}", name=f"q2p{p}") for p in range(4)]
        k2p8 = [acts.tile([128, NT], FP8, tag=f"k2p{p}", name=f"k2p{p}") for p in range(4)]
        # vaug8[pair]: [128 keys, 2 pair-member, H, 130] = v1(64) v2(64) ones(1) pad(1)
        vaug8 = [acts.tile([128, 2, H, 130], FP8, tag=f"va{p}", name=f"va{p}")
                 for p in range(NP)]
        o1n8 = [acts.tile([128, 2, NT], FP8, tag=f"o1n{p}", name=f"o1n{p}") for p in range(4)]
        o2n8 = [acts.tile([64, 2, NT], FP8, tag=f"o2n{p}", name=f"o2n{p}") for p in range(4)]
        rrb1 = acts.tile([128, NT], F32, tag="rrb1", name="rrb1")
        rrb2 = acts.tile([64, NT], F32, tag="rrb2", name="rrb2")

        for p in range(NP):
            # ones column (col 128 of each [.,.,h,130] block); memset pad too
            nc.gpsimd.memset(vaug8[p][:, :, :, 128:130], 1.0)

        pts = ctx.enter_context(tc.tile_pool(name="pts", bufs=10))
        rrow = ctx.enter_context(tc.tile_pool(name="rrow", bufs=2))
        inp = ctx.enter_context(tc.tile_pool(name="inp", bufs=1))
        yts = load(inp, "yT", [50, NT], F32R)
        xts = load(inp, "xT", [84, NT], F32R)

        # helpers parameterized by psum pool; psB uses [128,512] tiles so the
        # x-side projections can run concurrently with attention (psB+psS=6 banks)
        def phaseA(pool, w, src, dstf, dstb, bias):
            kdim = w.shape[0]
            for j in range(4):
                ps = pool.tile([128, NT], F32, tag="psA", name="psA")
                for nb in range(2):
                    nc.tensor.matmul(ps[:, nb * 512:(nb + 1) * 512],
                                     w[0:kdim, j * 128:(j + 1) * 128],
                                     src[0:kdim, nb * 512:(nb + 1) * 512],
                                     start=True, stop=True)
                nc.vector.tensor_scalar_add(dstf[j], ps, bias[:, j:j + 1])
                nc.vector.tensor_scalar_mul(dstb[:, j, :], ps, SX)

        def proj_q1(pool, h):
            # q1 folded projection for head h -> q1p8[h] (scale SQ1)
            for nb in range(2):
                ps = pool.tile([128, C], F32, tag="psB", name="psB")
                for g in range(2):
                    nc.tensor.matmul(ps,
                                     w1q8[g][:, :, h * 128:(h + 1) * 128],
                                     xcb8[:, 2 * g:2 * g + 2, nb * 512:(nb + 1) * 512],
                                     start=(g == 0), stop=(g == 1), perf_mode=DR)
                nc.vector.tensor_scalar_mul(q1p8[h][:, nb * 512:(nb + 1) * 512],
                                            ps, SQ1 / (SX * SW))

        def proj_kq2(pool, which, p):
            # which: 'k1' (x-side keys), 'k2' (y-side keys), 'q2' (raw y queries)
            wt, srcb = {"k1": (w1k8, xcb8), "k2": (w2k8, ycb8), "q2": (w2q8, ycb8)}[which]
            for nb in range(2):
                sl = slice(nb * 512, (nb + 1) * 512)
                ps = pool.tile([128, C], F32, tag="psB", name="psB")
                for g in range(2):
                    nc.tensor.matmul(ps,
                                     wt[g][:, :, p * 128:(p + 1) * 128],
                                     srcb[:, 2 * g:2 * g + 2, sl],
                                     start=(g == 0), stop=(g == 1), perf_mode=DR)
                if which == "k1":
                    nc.vector.tensor_scalar_mul(knew8[2 * p][0:64, sl], ps[0:64, :], SK / (SX * SW))
                    nc.vector.tensor_scalar_mul(knew8[2 * p + 1][0:64, sl], ps[64:128, :], SK / (SX * SW))
                elif which == "k2":
                    nc.vector.tensor_scalar_mul(knew8[2 * p][64:128, sl], ps[0:64, :], SK / (SX * SW))
                    nc.vector.tensor_scalar_mul(knew8[2 * p + 1][64:128, sl], ps[64:128, :], SK / (SX * SW))
                else:
                    nc.vector.tensor_scalar_mul(q2p8[p][:, sl], ps, SQ2 / (SX * SW))

        def proj_k2p(pool, p):
            # k2p' = M2^T-fold of k2 for heads 2p, 2p+1 -> k2p8[p] (scale SKP)
            for nb in range(2):
                sl = slice(nb * 512, (nb + 1) * 512)
                ps = pool.tile([128, C], F32, tag="psB", name="psB")
                for i in range(2):
                    nc.tensor.matmul(ps[64 * i:64 * i + 64, :],
                                     m2t8[64:128, :],
                                     knew8[2 * p + i][64:128, sl],
                                     start=True, stop=True)
                nc.vector.tensor_scalar_mul(k2p8[p][:, sl], ps, SKP / (SK * SM2))

        def proj_v(pool, src_is_x, kb):
            # v projection for key block kb -> vaug8 (scale 256)
            wt, srcb, lo = (w1v8, xcb8, 0) if src_is_x else (w2v8, ycb8, 64)
            ps = pool.tile([128, C], F32, tag="psB", name="psB")
            for g in range(2):
                nc.tensor.matmul(ps, srcb[:, 2 * g:2 * g + 2, kb * 128:(kb + 1) * 128],
                                 wt[g], start=(g == 0), stop=(g == 1), perf_mode=DR)
            nc.vector.tensor_copy(
                vaug8[kb // 2][:, kb % 2, :, lo:lo + 64],
                ps[:].rearrange("p (h d) -> p h d", h=H))

        def attn_branch2(psS, psO, h, pt2):
            hb, hp = 64 * (h % 2), h // 2
            for kb in range(KB):
                sps = psS.tile([128, NT], F32, tag="psS", name="psS")
                for nb in range(2):
                    nc.tensor.matmul(sps[:, nb * 512:(nb + 1) * 512],
                                     k2p8[hp][hb:hb + 64, kb * 128:(kb + 1) * 128],
                                     q2p8[hp][hb:hb + 64, nb * 512:(nb + 1) * 512],
                                     start=True, stop=True)
                nc.scalar.activation(pt2[kb // 2][:, kb % 2, :], sps, EXP, scale=EXPSCALE)

        def attn_branch1_s(psS, h, pt1):
            for kb in range(KB):
                sps = psS.tile([128, NT], F32, tag="psS", name="psS")
                for nb in range(2):
                    nc.tensor.matmul(sps[:, nb * 512:(nb + 1) * 512],
                                     knew8[h][:, kb * 128:(kb + 1) * 128],
                                     q1p8[h][:, nb * 512:(nb + 1) * 512],
                                     start=True, stop=True)
                nc.scalar.activation(pt1[kb // 2][:, kb % 2, :], sps, EXP, scale=EXPSCALE)

        def attn_o(psO, h, pt2, pt1):
            hp = h // 2
            ops2 = psO.tile([128, NT], F32, tag="psO2", name="psO2")
            ops1 = psO.tile([128, NT], F32, tag="psO1", name="psO1")
            for pr in range(NP):
                for nb in range(2):
                    nc.tensor.matmul(ops2[0:65, nb * 512:(nb + 1) * 512],
                                     vaug8[pr][:, :, h, 64:129],
                                     pt2[pr][:, :, nb * 512:(nb + 1) * 512],
                                     start=(pr == 0), stop=(pr == NP - 1), perf_mode=DR)
            for pr in range(NP):
                for nb in range(2):
                    nc.tensor.matmul(ops1[:, nb * 512:(nb + 1) * 512],
                                     vaug8[pr][:, :, h, 0:128],
                                     pt1[pr][:, :, nb * 512:(nb + 1) * 512],
                                     start=(pr == 0), stop=(pr == NP - 1), perf_mode=DR)
                    for i in range(2):
                        nc.tensor.matmul(ops2[96:97, nb * 512:(nb + 1) * 512],
                                         ones8[:, 0, 0:1],
                                         pt1[pr][:, i, nb * 512:(nb + 1) * 512],
                                         start=(pr == 0 and i == 0),
                                         stop=(pr == NP - 1 and i == 1),
                                         tile_position=(0, 96))
            # normalize: o_n = O * broadcast(1/r)   (o1n/o2n keep scale 256)
            # partition_broadcast requires partition-base-0 inputs
            rr2 = rrow.tile([1, NT], F32, tag="rr2", name="rr2")
            rr1 = rrow.tile([1, NT], F32, tag="rr1", name="rr1")
            nc.vector.reciprocal(rr2, ops2[64:65, :])
            nc.vector.reciprocal(rr1, ops2[96:97, :])
            nc.gpsimd.partition_broadcast(rrb2, rr2)
            nc.vector.tensor_mul(o2n8[hp][:, h % 2, :], ops2[0:64, :], rrb2)
            nc.gpsimd.partition_broadcast(rrb1, rr1)
            nc.vector.tensor_mul(o1n8[hp][:, h % 2, :], ops1, rrb1)

        pt = {}
        for h in range(H):
            pt[h] = ([pts.tile([128, 2, NT], FP8, tag="pt", name="pt") for _ in range(NP)],
                     [pts.tile([128, 2, NT], FP8, tag="pt", name="pt") for _ in range(NP)])

        # phase 1 [psA 4 + psB 2 banks]: y-side, A-x, v
        with tc.tile_pool(name="psA", bufs=2, space="PSUM") as psA, \
             tc.tile_pool(name="psB", bufs=2, space="PSUM") as psB:
            phaseA(psA, w2, yts, yc, ycb8, bp2)
            for p in range(4):
                proj_kq2(psA, "k2", p)
            for p in range(4):
                proj_kq2(psA, "q2", p)
                proj_k2p(psA, p)
            phaseA(psA, w1, xts, xc, xcb8, bp1)
            for kb in range(KB):
                proj_v(psB, False, kb)
                proj_v(psB, True, kb)

        # phase 2 [psB2 2 + psS 4]: S2(h0) while x-side k1/q1 run, then S1(h0)
        with tc.tile_pool(name="psS", bufs=2, space="PSUM") as psS:
            with tc.tile_pool(name="psB2", bufs=2, space="PSUM") as psB2:
                attn_branch2(psS, None, 0, pt[0][0])
                for p in range(4):
                    proj_kq2(psB2, "k1", p)
                for h in range(H):
                    proj_q1(psB2, h)
                attn_branch1_s(psS, 0, pt[0][1])

            # phase 3 [psS 4 + psO 4]: O(h0), heads 1..7
            with tc.tile_pool(name="psO", bufs=1, space="PSUM") as psO:
                attn_o(psO, 0, pt[0][0], pt[0][1])
                for h in range(1, H):
                    attn_branch2(psS, psO, h, pt[h][0])
                    attn_branch1_s(psS, h, pt[h][1])
                    attn_o(psO, h, pt[h][0], pt[h][1])

        # phase 4 [psE 4]: output projections + residuals
        with tc.tile_pool(name="psE", bufs=2, space="PSUM") as psE, \
             tc.tile_pool(name="outp", bufs=3) as outp:
            for (wp, on, res, q1off, rowoff) in (
                    (wp2p8, o2n8, yc, 4, C),
                    (wp1p8, o1n8, xc, 0, 0)):
                for j in range(4):
                    zps = psE.tile([128, NT], F32, tag="psE", name="psE")
                    for nb in range(2):
                        sl = slice(nb * 512, (nb + 1) * 512)
                        for p in range(4):
                            nc.tensor.matmul(zps[:, sl],
                                             wp[p][:, :, j * 128:(j + 1) * 128],
                                             on[p][:, :, sl],
                                             start=(p == 0), stop=False, perf_mode=DR)
                        nc.tensor.matmul(zps[:, sl], ident8,
                                         q1p8[q1off + j][:, sl],
                                         start=False, stop=True)
                    of = outp.tile([128, NT], F32, tag="of", name="of")
                    nc.vector.scalar_tensor_tensor(
                        out=of, in0=zps, scalar=1.0 / SE, in1=res[j], op0=MULT, op1=ADD)
                    nc.sync.dma_start(
                        out=outT[rowoff + j * 128:rowoff + (j + 1) * 128, :], in_=of)


class _Runner:
    def __init__(self):
        import jax
        from jax.sharding import Mesh, PartitionSpec
        from jax.experimental.shard_map import shard_map

        nc = bacc.Bacc("TRN2", target_bir_lowering=False, debug=False,
                       num_devices=N_CORES)
        _build(nc)
        nc.compile()
        self.nc = nc

        bass2jax.install_neuronx_cc_hook()
        part_name = nc.partition_id_tensor.name if nc.partition_id_tensor else None
        in_names, out_names, out_avals, self.zero_shapes = [], [], [], []
        for alloc in nc.m.functions[0].allocations:
            if not isinstance(alloc, mybir.MemoryLocationSet):
                continue
            name = alloc.memorylocations[0].name
            if alloc.kind == "ExternalInput":
                if name != part_name:
                    in_names.append(name)
            elif alloc.kind == "ExternalOutput":
                out_names.append(name)
                shape = tuple(alloc.tensor_shape)
                dtype = mybir.dt.np(alloc.dtype)
                out_avals.append(jax.core.ShapedArray(shape, dtype))
                self.zero_shapes.append((shape, dtype))
        self.in_names, self.out_names, self.out_avals = in_names, out_names, out_avals
        n_params, n_outs = len(in_names), len(out_avals)
        all_names = in_names + out_names + ([part_name] if part_name else [])

        def _bodyfn(*args):
            operands = list(args)
            if part_name:
                operands.append(bass2jax.partition_id_tensor())
            outs = bass2jax._bass_exec_p.bind(
                *operands, out_avals=tuple(out_avals), in_names=tuple(all_names),
                out_names=tuple(out_names), lowering_input_output_aliases=(),
                sim_require_finite=True, sim_require_nnan=True, nc=nc)
            return tuple(outs)

        devices = jax.devices()[:N_CORES]
        mesh = Mesh(np.asarray(devices), ("core",))
        self._fn = jax.jit(
            shard_map(_bodyfn, mesh=mesh,
                      in_specs=(PartitionSpec("core"),) * (n_params + n_outs),
                      out_specs=(PartitionSpec("core"),) * n_outs,
                      check_rep=False),
            donate_argnums=tuple(range(n_params, n_params + n_outs)),
            keep_unused=True)
        self._jax = jax

    def __call__(self, in_maps):
        concat_in = [np.concatenate([m[n] for m in in_maps], axis=0)
                     for n in self.in_names]
        zeros = [np.zeros((N_CORES * s[0], *s[1:]), d) for s, d in self.zero_shapes]
        outs = self._fn(*concat_in, *zeros)
        self._jax.block_until_ready(outs)
        return [
            {n: np.asarray(outs[i]).reshape(N_CORES, *self.out_avals[i].shape)[c]
             for i, n in enumerate(self.out_names)}
            for c in range(N_CORES)
        ]


_RUNNER = None


def _get_runner():
    global _RUNNER
    if _RUNNER is None:
        _RUNNER = _Runner()
    return _RUNNER


def _to8(a, scale):
    return (np.asarray(a, np.float64) * scale).astype(ml_dtypes.float8_e4m3)


def _ileave_c(W, scale):
    # [512, M] -> two [128, 2, M] tiles: (g)[p, i, m] = W[g*256 + i*128 + p, m]
    W = np.asarray(W, np.float64) * scale
    out = []
    for g in range(2):
        out.append(np.stack([W[(2 * g) * 128:(2 * g + 1) * 128],
                             (W[(2 * g + 1) * 128:(2 * g + 2) * 128])], axis=1)
                   .astype(ml_dtypes.float8_e4m3))
    return out


def _prep_in_maps(inputs):
    f32 = np.float32
    x = np.asarray(inputs["x"], f32)
    y = np.asarray(inputs["y"], f32)
    Wqkv1 = np.asarray(inputs["Wqkv1"], np.float64)
    Wqkv2 = np.asarray(inputs["Wqkv2"], np.float64)
    Wq1 = np.asarray(inputs["Wq1"], np.float64)
    Wq2 = np.asarray(inputs["Wq2"], np.float64)
    Wk2 = np.asarray(inputs["Wk2"], np.float64)

    # fold per-head q1 projection: w1q[:, h*128+e] = sum_d Wqkv1[:, h*64+d] Wq1[d, e]
    w1q = np.zeros((C, 1024), np.float64)
    for h in range(H):
        w1q[:, h * D2:(h + 1) * D2] = Wqkv1[:, h * D:(h + 1) * D] @ Wq1
    m2 = Wq2 @ Wk2.T      # [64, 64]; S2 = q2 @ m2 @ k2^T

    shared = {
        "W1": np.ascontiguousarray(inputs["W1"], f32),
        "W2": np.ascontiguousarray(inputs["W2"], f32),
        "m2t8": _to8(m2.T, SM2),
        "ident8": _to8(np.eye(D2), SIDENT),
        "bp1": np.ascontiguousarray(inputs["bp1"], f32),
        "bp2": np.ascontiguousarray(inputs["bp2"], f32),
    }
    for g, t in enumerate(_ileave_c(Wqkv1[:, 1024:1536], SW)):
        shared[f"w1v8_{g}"] = t
    for g, t in enumerate(_ileave_c(Wqkv2[:, 1024:1536], SW)):
        shared[f"w2v8_{g}"] = t
    for g, t in enumerate(_ileave_c(Wqkv1[:, 512:1024], SW)):
        shared[f"w1k8_{g}"] = t
    for g, t in enumerate(_ileave_c(Wqkv2[:, 512:1024], SW)):
        shared[f"w2k8_{g}"] = t
    for g, t in enumerate(_ileave_c(w1q, SW)):
        shared[f"w1q8_{g}"] = t
    for g, t in enumerate(_ileave_c(Wqkv2[:, 0:512], SW)):
        shared[f"w2q8_{g}"] = t
    # output projections, head-pair interleaved along contraction
    Wp1 = np.asarray(inputs["Wp1"], np.float64)   # [1024, 512], rows h*128+c
    Wp2 = np.asarray(inputs["Wp2"], np.float64)   # [512, 512], rows h*64+c
    for p in range(4):
        shared[f"wp1p8_{p}"] = np.stack(
            [Wp1[(2 * p) * 128:(2 * p + 1) * 128] * SWP,
             Wp1[(2 * p + 1) * 128:(2 * p + 2) * 128] * SWP],
            axis=1).astype(ml_dtypes.float8_e4m3)
        shared[f"wp2p8_{p}"] = np.stack(
            [Wp2[(2 * p) * 64:(2 * p + 1) * 64] * SWP,
             Wp2[(2 * p + 1) * 64:(2 * p + 2) * 64] * SWP],
            axis=1).astype(ml_dtypes.float8_e4m3)

    in_maps = []
    for b in range(N_CORES):
        m = dict(shared)
        m["xT"] = np.ascontiguousarray(x[b].T)
        m["yT"] = np.ascontiguousarray(y[b].T)
        in_maps.append(m)
    return in_maps


def kernel(**inputs):
    runner = _get_runner()
    in_maps = _prep_in_maps(inputs)
    results = runner(in_maps)
    out = np.stack([results[b]["outT"].T for b in range(N_CORES)], axis=0)
    return out.astype(np.float32)


if __name__ == "__main__":
    rng = np.random.default_rng(0)
    s = 0.02
    inputs = {
        "x": rng.standard_normal((8, NT, 84), dtype=np.float32),
        "y": rng.standard_normal((8, NT, 50), dtype=np.float32),
        "W1": rng.standard_normal((84, C), dtype=np.float32) * s,
        "W2": rng.standard_normal((50, C), dtype=np.float32) * s,
        "Wqkv1": rng.standard_normal((C, 1536), dtype=np.float32) * s,
        "Wqkv2": rng.standard_normal((C, 1536), dtype=np.float32) * s,
        "Wq1": rng.standard_normal((D, D2), dtype=np.float32) * s,
        "Wq2": rng.standard_normal((D, D2), dtype=np.float32) * s,
        "Wk2": rng.standard_normal((D, D2), dtype=np.float32) * s,
        "Wp1": rng.standard_normal((1024, C), dtype=np.float32) * s,
        "bp1": np.zeros(C, np.float32),
        "Wp2": rng.standard_normal((C, C), dtype=np.float32) * s,
        "bp2": np.zeros(C, np.float32),
    }
    out = kernel(**inputs)
    print("out", out.shape, out.dtype, np.abs(out).max())
